# revision 5
# baseline (speedup 1.0000x reference)
"""Trainium2 Bass kernel for nn_Bottleneck_CSA_ConvBlock.

Computation (per image, C=64, H=W=160):
    y  = silu(bn1(conv3x3(x, w1)))
    fq = conv3x3(y, wq); fk = conv3x3(y, wk); fv = conv3x3(y, wv)
    k_sum = fk.sum(ch, h); f_scores[c] = scale * sum_hw fq[c,h,w]*k_sum[w]
    scores = softmax_c(f_scores)
    out = x + relu(bn2(scores*fv + y))

Key algebraic reduction: f_scores and k_sum are linear functionals of the
column sums of y (plus its first/last rows), so fq/fk are never materialized.
Only two full convs run (x->y and y->fv), each lowered to 9 shifted bf16
matmuls accumulating in PSUM over a zero-padded image layout. The two images
of each core sit on partition halves and run concurrently in opposite
quadrants of the PE array (tile_position packing). fp32 accumulation in PSUM;
the final residual add uses the exact fp32 x.

Sharding: pure data parallelism, 2 images per core across 8 cores.
"""

import numpy as np

C = 64
H = W = 160
HP = WP = 162          # padded
IMG = H * W            # 25600
PIMG = HP * WP         # 26244
BN_EPS = 1e-5

_CACHED = {}


def _build_nc():
    import concourse.bass as bass
    import concourse.tile as tile
    from concourse import bacc, mybir
    from concourse.masks import make_identity

    dt = mybir.dt
    AF = mybir.ActivationFunctionType
    AX = mybir.AxisListType
    f32 = dt.float32
    bf16 = dt.bfloat16

    nc = bacc.Bacc("TRN2", target_bir_lowering=False, debug=False, num_devices=8)

    x_d = nc.dram_tensor("x", [128, IMG], f32, kind="ExternalInput")
    xb_d = nc.dram_tensor("xbf", [128, IMG], bf16, kind="ExternalInput")
    w1_d = nc.dram_tensor("w1t", [128, 9, 64], bf16, kind="ExternalInput")
    wv_d = nc.dram_tensor("wvt", [128, 9, 64], bf16, kind="ExternalInput")
    wq_d = nc.dram_tensor("wqt", [128, 9, 65], bf16, kind="ExternalInput")
    bn1s_d = nc.dram_tensor("bn1s", [128, 1], f32, kind="ExternalInput")
    bn1b_d = nc.dram_tensor("bn1b", [128, 1], f32, kind="ExternalInput")
    bn2s_d = nc.dram_tensor("bn2s", [128, 1], f32, kind="ExternalInput")
    bn2b_d = nc.dram_tensor("bn2b", [128, 1], f32, kind="ExternalInput")
    out_d = nc.dram_tensor("out", [128, IMG], f32, kind="ExternalOutput")

    # output row blocks: (first interior row r0 in padded coords, n rows)
    blocks = [(1 + 3 * i, 3) for i in range(53)] + [(160, 1)]
    NCH = 26            # x chunk capacity in padded rows
    CH_LEN = 1 + NCH * WP + 1

    with tile.TileContext(nc) as tc:
        ctx_lp = nc.allow_low_precision("bf16 matmul path; fp32 PSUM accumulation")
        ctx_lp.__enter__()
        with (
            tc.tile_pool(name="const", bufs=1) as const,
            tc.tile_pool(name="ybuf", bufs=1) as ybuf,
            tc.tile_pool(name="small", bufs=1) as small,
        ):
            w1_sb = const.tile([128, 9, 64], bf16)
            nc.sync.dma_start(out=w1_sb[:], in_=w1_d.ap())
            wv_sb = const.tile([128, 9, 64], bf16)
            nc.sync.dma_start(out=wv_sb[:], in_=wv_d.ap())
            wq_sb = const.tile([128, 9, 65], bf16)
            nc.sync.dma_start(out=wq_sb[:], in_=wq_d.ap())
            bn1s = const.tile([128, 1], f32)
            nc.sync.dma_start(out=bn1s[:], in_=bn1s_d.ap())
            bn1b = const.tile([128, 1], f32)
            nc.sync.dma_start(out=bn1b[:], in_=bn1b_d.ap())
            bn2s = const.tile([128, 1], f32)
            nc.sync.dma_start(out=bn2s[:], in_=bn2s_d.ap())
            bn2b = const.tile([128, 1], f32)
            nc.sync.dma_start(out=bn2b[:], in_=bn2b_d.ap())
            ident = const.tile([128, 128], f32)
            make_identity(nc, ident[:])
            ones_sb = const.tile([128, 64], bf16)
            nc.vector.memset(ones_sb[:], 1.0)

            # persistent padded y (bf16): free index = 1 + r*WP + c, 1-elem slack
            y_pad = ybuf.tile([128, 1 + PIMG + 1], bf16)
            y3 = y_pad[:, 1:1 + PIMG].rearrange("p (r c) -> p r c", c=WP)
            nc.vector.memset(y_pad[:, 0:1], 0.0)
            nc.vector.memset(y_pad[:, 1 + PIMG:1 + PIMG + 1], 0.0)
            nc.vector.memset(y3[:, 0, :], 0.0)
            nc.vector.memset(y3[:, HP - 1, :], 0.0)
            nc.vector.memset(y3[:, 1:HP - 1, 0:1], 0.0)
            nc.vector.memset(y3[:, 1:HP - 1, WP - 1:WP], 0.0)

            C_sb = small.tile([128, WP], bf16)       # column sums of y
            CmL = small.tile([128, WP], bf16)        # C - last row
            CmF = small.tile([128, WP], bf16)        # C - first row
            q0s = small.tile([65, 160], bf16)
            q1s = small.tile([65, 160], bf16)
            t0s = small.tile([64, 160], f32)
            t1s = small.tile([64, 160], f32)
            fs0 = small.tile([64, 1], f32)
            fs1 = small.tile([64, 1], f32)
            frow = small.tile([1, 128], f32)
            srow = small.tile([1, 128], f32)
            mx = small.tile([1, 1], f32, tag="mx")
            sm = small.tile([1, 1], f32, tag="sm")
            rs = small.tile([1, 1], f32, tag="rs")
            scores = small.tile([128, 1], f32)

            xap = x_d.ap()
            xbap = xb_d.ap()

            # ---------------- pass 1: conv1 -> y, column sums ----------------
            with (
                tc.tile_pool(name="chunks", bufs=3) as chunks,
                tc.tile_pool(name="ps1", bufs=4, space="PSUM") as ps1,
                tc.tile_pool(name="csum", bufs=2) as csum,
            ):
                for k in range(7):
                    pr0 = 24 * k
                    nrows = 26 if k < 6 else 18
                    ch = chunks.tile([128, CH_LEN], bf16, tag="ch")
                    ch3 = ch[:, 1:1 + nrows * WP].rearrange("p (r c) -> p r c", c=WP)
                    # zero pads: slack cells, left/right cols, boundary rows
                    nc.vector.memset(ch[:, 0:1], 0.0)
                    nc.vector.memset(ch[:, 1 + nrows * WP:1 + nrows * WP + 1], 0.0)
                    nc.vector.memset(ch3[:, :, 0:1], 0.0)
                    nc.vector.memset(ch3[:, :, WP - 1:WP], 0.0)
                    if k == 0:
                        nc.vector.memset(ch3[:, 0, :], 0.0)
                        ir0, nir, l0 = 0, 25, 1
                    elif k < 6:
                        ir0, nir, l0 = pr0 - 1, 26, 0
                    else:
                        nc.vector.memset(ch3[:, 17, :], 0.0)
                        ir0, nir, l0 = pr0 - 1, 17, 0
                    nc.sync.dma_start(
                        out=ch3[:, l0:l0 + nir, 1:1 + W],
                        in_=xbap[:, ir0 * W:(ir0 + nir) * W].rearrange(
                            "p (r c) -> p r c", c=W),
                    )

                    for r0, nr in blocks:
                        if (r0 - 1) // 24 != k:
                            continue
                        lr = r0 - pr0
                        N = nr * WP
                        ps = ps1.tile([128, 3 * WP], f32, tag="ps")
                        for k9 in range(9):
                            dy, dx = divmod(k9, 3)
                            off = 1 + (lr + dy - 1) * WP + (dx - 1)
                            for img in range(2):
                                b = 64 * img
                                nc.tensor.matmul(
                                    ps[b:b + 64, :N],
                                    w1_sb[b:b + 64, k9, :],
                                    ch[b:b + 64, off:off + N],
                                    start=(k9 == 0), stop=(k9 == 8),
                                    tile_position=(b, b),
                                )
                        ps3 = ps[:, :N].rearrange("p (r c) -> p r c", c=WP)
                        nc.scalar.activation(
                            out=y3[:, r0:r0 + nr, 1:1 + W],
                            in_=ps3[:, :, 1:1 + W],
                            func=AF.Silu, bias=bn1b[:], scale=bn1s[:],
                        )

                    # partial column sums over this chunk's freshly written rows
                    yr0 = 24 * k + 1
                    ynr = 24 if k < 6 else 16
                    part = csum.tile([128, WP], f32, tag="part")
                    nc.vector.reduce_sum(
                        part[:],
                        y3[:, yr0:yr0 + ynr, :].rearrange("p r c -> p c r"),
                        axis=AX.X,
                    )
                    if k == 0:
                        nc.vector.tensor_copy(C_sb[:], part[:])
                    else:
                        nc.vector.tensor_add(C_sb[:], C_sb[:], part[:])

            # ---------------- scores (small path) ----------------
            with tc.tile_pool(name="ps_s", bufs=2, space="PSUM") as pss:
                nc.vector.tensor_sub(CmL[:], C_sb[:], y3[:, H, :])
                nc.vector.tensor_sub(CmF[:], C_sb[:], y3[:, 1, :])
                s_of = {0: CmL, 1: C_sb, 2: CmF}

                qp0 = pss.tile([65, 160], f32, tag="qp")
                qp1 = pss.tile([65, 160], f32, tag="qp")
                for k9 in range(9):
                    dy, dx = divmod(k9, 3)
                    src = s_of[dy]
                    nc.tensor.matmul(
                        qp0[:, :], wq_sb[0:64, k9, :], src[0:64, dx:dx + 160],
                        start=(k9 == 0), stop=(k9 == 8), tile_position=(0, 0),
                    )
                for k9 in range(9):
                    dy, dx = divmod(k9, 3)
                    src = s_of[dy]
                    nc.tensor.matmul(
                        qp1[:, :], wq_sb[64:128, k9, :], src[64:128, dx:dx + 160],
                        start=(k9 == 0), stop=(k9 == 8), tile_position=(64, 0),
                    )
                nc.vector.tensor_copy(q0s[:], qp0[:])
                nc.vector.tensor_copy(q1s[:], qp1[:])

                # broadcast k_sum row (partition 64) across 64 partitions
                bc0 = pss.tile([64, 160], f32, tag="bc")
                bc1 = pss.tile([64, 160], f32, tag="bc")
                nc.tensor.matmul(bc0[:, :], ones_sb[64:65, :], q0s[64:65, :],
                                 start=True, stop=True, tile_position=(64, 0))
                nc.tensor.matmul(bc1[:, :], ones_sb[64:65, :], q1s[64:65, :],
                                 start=True, stop=True, tile_position=(64, 0))
                nc.vector.tensor_mul(t0s[:], q0s[0:64, :], bc0[:])
                nc.vector.tensor_mul(t1s[:], q1s[0:64, :], bc1[:])
                nc.vector.reduce_sum(fs0[:], t0s[:], axis=AX.X)
                nc.vector.reduce_sum(fs1[:], t1s[:], axis=AX.X)

                tr0 = pss.tile([1, 64], f32, tag="tr")
                tr1 = pss.tile([1, 64], f32, tag="tr")
                nc.tensor.transpose(tr0[:], fs0[:], ident[0:64, 0:64])
                nc.tensor.transpose(tr1[:], fs1[:], ident[0:64, 0:64])
                nc.vector.tensor_copy(frow[0:1, 0:64], tr0[:])
                nc.vector.tensor_copy(frow[0:1, 64:128], tr1[:])

                for img in range(2):
                    seg = frow[0:1, 64 * img:64 * img + 64]
                    oseg = srow[0:1, 64 * img:64 * img + 64]
                    nc.vector.reduce_max(mx[:], seg, axis=AX.X, negate=True)
                    nc.scalar.activation(out=oseg, in_=seg, func=AF.Exp,
                                         bias=mx[:], scale=1.0)
                    nc.vector.reduce_sum(sm[:], oseg, axis=AX.X)
                    nc.vector.reciprocal(rs[:], sm[:])
                    nc.vector.tensor_scalar_mul(oseg, oseg, rs[:])

                psc = pss.tile([128, 1], f32, tag="psc")
                nc.tensor.transpose(psc[:], srow[:], ident[0:1, 0:1])
                nc.vector.tensor_copy(scores[:], psc[:])

            # ---------------- pass 2: conv_v -> epilogue -> out ----------------
            with (
                tc.tile_pool(name="ps2", bufs=4, space="PSUM") as ps2,
                tc.tile_pool(name="epi", bufs=3) as epi,
            ):
                for r0, nr in blocks:
                    N = nr * WP
                    M = nr * W
                    ps = ps2.tile([128, 3 * WP], f32, tag="ps")
                    for k9 in range(9):
                        dy, dx = divmod(k9, 3)
                        off = 1 + (r0 + dy - 1) * WP + (dx - 1)
                        for img in range(2):
                            b = 64 * img
                            nc.tensor.matmul(
                                ps[b:b + 64, :N],
                                wv_sb[b:b + 64, k9, :],
                                y_pad[b:b + 64, off:off + N],
                                start=(k9 == 0), stop=(k9 == 8),
                                tile_position=(b, b),
                            )
                    ps3 = ps[:, :N].rearrange("p (r c) -> p r c", c=WP)
                    u = epi.tile([128, 3 * W], bf16, tag="u")
                    nc.scalar.mul(u[:, :M], ps3[:, :, 1:1 + W], scores[:])
                    u2 = epi.tile([128, 3 * W], bf16, tag="u2")
                    nc.vector.tensor_add(u2[:, :M], u[:, :M],
                                         y3[:, r0:r0 + nr, 1:1 + W])
                    rt = epi.tile([128, 3 * W], bf16, tag="rt")
                    nc.scalar.activation(out=rt[:, :M], in_=u2[:, :M],
                                         func=AF.Relu, bias=bn2b[:], scale=bn2s[:])
                    xt = epi.tile([128, 3 * W], f32, tag="xt")
                    nc.sync.dma_start(out=xt[:, :M],
                                      in_=xap[:, (r0 - 1) * W:(r0 - 1) * W + M])
                    ot = epi.tile([128, 3 * W], f32, tag="ot")
                    nc.vector.tensor_add(ot[:, :M], rt[:, :M], xt[:, :M])
                    nc.sync.dma_start(out=out_d.ap()[:, (r0 - 1) * W:(r0 - 1) * W + M],
                                      in_=ot[:, :M])
        ctx_lp.__exit__(None, None, None)
    nc.compile()
    return nc


def _get_nc():
    if "nc" not in _CACHED:
        _CACHED["nc"] = _build_nc()
    return _CACHED["nc"]


def _prep_weights(w_cv1, wq, wk, wv, g1, b1, m1, v1, g2, b2, m2, v2):
    import ml_dtypes
    bf = ml_dtypes.bfloat16

    def wt(w):  # [co, ci, ky, kx] -> [ci, 9, co], doubled over partitions
        t = np.ascontiguousarray(w.transpose(1, 2, 3, 0).reshape(C, 9, C))
        return np.ascontiguousarray(np.concatenate([t, t], axis=0).astype(bf))

    w1t = wt(w_cv1)
    wvt = wt(wv)

    scale = 1.0 / (float(W) ** 0.5 * float(H) * float(H))
    q = wq.transpose(1, 2, 3, 0).reshape(C, 9, C) * scale    # [j, 9, c]
    ks = wk.sum(axis=0).reshape(C, 9, 1)                     # [j, 9, 1]
    qa = np.concatenate([q, ks], axis=2)                     # [j, 9, 65]
    wqt = np.ascontiguousarray(np.concatenate([qa, qa], axis=0).astype(bf))

    s1 = (g1 / np.sqrt(v1 + BN_EPS)).astype(np.float32)
    b1p = (b1 - m1 * s1).astype(np.float32)
    s2 = (g2 / np.sqrt(v2 + BN_EPS)).astype(np.float32)
    b2p = (b2 - m2 * s2).astype(np.float32)

    def dup(v):
        return np.ascontiguousarray(
            np.concatenate([v, v]).reshape(128, 1).astype(np.float32))

    return dict(w1t=w1t, wvt=wvt, wqt=wqt,
                bn1s=dup(s1), bn1b=dup(b1p), bn2s=dup(s2), bn2b=dup(b2p))


def kernel(x, w_cv1, g1, b1, m1, v1, wq, wk, wv, g2, b2, m2, v2):
    import ml_dtypes
    from concourse.bass_utils import run_bass_kernel_spmd

    x = np.asarray(x, dtype=np.float32)
    consts = _prep_weights(
        np.asarray(w_cv1, np.float32), np.asarray(wq, np.float32),
        np.asarray(wk, np.float32), np.asarray(wv, np.float32),
        np.asarray(g1, np.float32), np.asarray(b1, np.float32),
        np.asarray(m1, np.float32), np.asarray(v1, np.float32),
        np.asarray(g2, np.float32), np.asarray(b2, np.float32),
        np.asarray(m2, np.float32), np.asarray(v2, np.float32))
    nc = _get_nc()
    in_maps = []
    for i in range(8):
        xi = np.ascontiguousarray(x[2 * i:2 * i + 2].reshape(128, IMG))
        m = {"x": xi, "xbf": np.ascontiguousarray(xi.astype(ml_dtypes.bfloat16))}
        m.update(consts)
        in_maps.append(m)
    res = run_bass_kernel_spmd(nc, in_maps, core_ids=list(range(8)))
    outs = [r["out"].reshape(2, C, H, W) for r in res.results]
    return np.concatenate(outs, axis=0).astype(np.float32)
